# revision 18
# baseline (speedup 1.0000x reference)
"""Trainium2 Bass kernel for nn_CGFA (cross-graph feature aggregation / graph matching).

Pure data parallel over 8 NeuronCores: batch B=4096 -> 512 pairs per core.
Per core the batch is processed in tiles of G=8 pairs (4 stacks of 2 pairs,
64 nodes each, parity-packed on the 128 partitions).

Host-side prep (layout/dtype only): emb is uploaded pre-transposed
(feature-major) and A pre-transposed, both bf16, so the kernel never
transposes inputs on the PE.

3-stage software pipeline: A(t+2) loads+gconv | B(t+1) affinity+softmax |
C(t) cross-graph update + pooling, so every engine keeps ~1 tile of slack.

Only activation funcs from the shared "exp_and_others" table are used
(Exp/Relu/Tanh/Identity/Copy) to avoid ACT_TABLE_LOAD thrash; sigmoid is
computed as 0.5 + 0.5*tanh(x/2) with the affine folded into the pooling
combine.
"""

import os
import sys

STAGE = int(os.environ.get("CGFA_STAGE", "6"))

sys.path.insert(0, "/opt/trn_rl_repo")

import numpy as np

from concourse import bass, bacc
import concourse.mybir as mybir
from concourse.bass_utils import run_bass_kernel_spmd
from concourse.tile import TileContext

F32 = mybir.dt.float32
BF = mybir.dt.bfloat16
AF = mybir.ActivationFunctionType
ALU = mybir.AluOpType
AX = mybir.AxisListType

B, N, D = 4096, 64, 128
NCORES = 8
G = 8          # pairs per tile
ST = G // 2    # 128-partition stacks per side


def _emit(nc, n_pairs, has_ba=False):
    NT = n_pairs // G

    # ---- DRAM I/O ----
    dET1 = nc.dram_tensor("embT_src", [NT, D, G * N], BF, kind="ExternalInput").ap()
    dET2 = nc.dram_tensor("embT_dst", [NT, D, G * N], BF, kind="ExternalInput").ap()
    dAT1 = nc.dram_tensor("AT_src", [n_pairs, N, N], BF, kind="ExternalInput").ap()
    dAT2 = nc.dram_tensor("AT_dst", [n_pairs, N, N], BF, kind="ExternalInput").ap()
    dWa = nc.dram_tensor("Wa", [D, D], BF, kind="ExternalInput").ap()
    dWu = nc.dram_tensor("Wu", [D, D], BF, kind="ExternalInput").ap()
    dAff = nc.dram_tensor("Aff", [D, D], BF, kind="ExternalInput").ap()
    dWct = nc.dram_tensor("Wct", [D, D], BF, kind="ExternalInput").ap()
    dWcb = nc.dram_tensor("Wcb", [D, D], BF, kind="ExternalInput").ap()
    dWp1 = nc.dram_tensor("Wp1", [D, D], F32, kind="ExternalInput").ap()
    dWp2 = nc.dram_tensor("Wp2", [D, D], F32, kind="ExternalInput").ap()
    dbu = nc.dram_tensor("bu_col", [D, 1], F32, kind="ExternalInput").ap()
    dbc = nc.dram_tensor("bc_col", [D, 1], F32, kind="ExternalInput").ap()
    dba = nc.dram_tensor("ba_row", [1, D], F32, kind="ExternalInput").ap()
    dIb = nc.dram_tensor("ident_bf", [128, 128], BF, kind="ExternalInput").ap()
    dI = nc.dram_tensor("ident", [128, 128], F32, kind="ExternalInput").ap()
    dg1 = nc.dram_tensor("g1", [n_pairs, D], F32, kind="ExternalOutput").ap()
    dg2 = nc.dram_tensor("g2", [n_pairs, D], F32, kind="ExternalOutput").ap()

    with TileContext(nc) as tc:
        with (
            tc.tile_pool(name="const", bufs=1) as cpool,
            tc.tile_pool(name="work", bufs=3) as wpool,
            tc.tile_pool(name="keep", bufs=4) as kpool,     # A-outputs live A(t)..C(t)
            tc.tile_pool(name="zb", bufs=2) as zpool,       # pre-zeroed block-diag tiles
            tc.tile_pool(name="psum", bufs=6, space="PSUM") as ppool,
            tc.tile_pool(name="psums", bufs=2, space="PSUM") as spool,
        ):
            # ---- constants ----
            Wa = cpool.tile([128, 128], BF, tag="Wa")
            Wu = cpool.tile([128, 128], BF, tag="Wu")
            Aff = cpool.tile([128, 128], BF, tag="Aff")
            Wct = cpool.tile([128, 128], BF, tag="Wct")
            Wcb = cpool.tile([128, 128], BF, tag="Wcb")
            Wp1 = cpool.tile([128, 128], F32, tag="Wp1")
            Wp2 = cpool.tile([128, 128], F32, tag="Wp2")
            Ib = cpool.tile([128, 128], BF, tag="Ib")
            I = cpool.tile([128, 128], F32, tag="I")
            bu = cpool.tile([128, 1], F32, tag="bu")
            bc = cpool.tile([128, 1], F32, tag="bc")
            ones_col = cpool.tile([128, 1], BF, tag="ones_col")
            half_row = cpool.tile([1, 128], BF, tag="half_row")
            for tile_, src in (
                (Wa, dWa), (Wu, dWu), (Aff, dAff), (Wct, dWct), (Wcb, dWcb),
                (Wp1, dWp1), (Wp2, dWp2), (Ib, dIb), (I, dI), (bu, dbu), (bc, dbc),
            ):
                nc.sync.dma_start(out=tile_[:], in_=src)
            nc.gpsimd.memset(ones_col[:], 1.0)
            nc.gpsimd.memset(half_row[:], 0.5)
            if has_ba:
                ba_row = cpool.tile([1, 128], F32, tag="ba_row")
                nc.sync.dma_start(out=ba_row[:], in_=dba)
                ba_full = cpool.tile([128, 128], F32, tag="ba_full")
                nc.gpsimd.partition_broadcast(ba_full[:], ba_row[:])

            # whole-core pooled-output accumulators (written tile by tile)
            gacc1 = cpool.tile([128, n_pairs], F32, tag="gacc1")
            gacc2 = cpool.tile([128, n_pairs], F32, tag="gacc2")

            def stage_a_dma(t):
                """Issue all of tile t's input DMAs early."""
                out = {}
                for side, dET, dAT in ((1, dET1, dAT1), (2, dET2, dAT2)):
                    s = str(side)
                    eT = wpool.tile([128, ST, 128], BF, tag="eT" + s)
                    nc.sync.dma_start(out=eT[:], in_=dET[t].rearrange("d (s c) -> d s c", s=ST))
                    AT = zpool.tile([128, ST, 128], BF, tag="AT" + s)
                    if t < 2:
                        nc.gpsimd.memset(AT[:], 0.0)
                    nc.sync.dma_start(
                        out=AT[0:64, :, 0:64],
                        in_=dAT[t * G:(t + 1) * G:2].rearrange("g j i -> j g i"),
                    )
                    nc.sync.dma_start(
                        out=AT[64:128, :, 64:128],
                        in_=dAT[t * G + 1:(t + 1) * G:2].rearrange("g j i -> j g i"),
                    )
                    out[side] = (eT, AT)
                return out

            def phase_a_side(eT, AT, t, side):
                """gconv for one side; returns (eTe, e_n)."""
                s = str(side)
                # column sums of A (= row sums of A^T) -> 1/c, fold into A^T
                c = wpool.tile([128, ST], F32, tag="c" + s)
                nc.vector.reduce_sum(c[:], AT[:], axis=AX.X)
                nc.vector.tensor_scalar_max(c[:], c[:], 1e-12)
                rc = wpool.tile([128, ST], F32, tag="rc" + s)
                nc.vector.reciprocal(rc[:], c[:])
                AnT = wpool.tile([128, ST, 128], BF, tag="AnT" + s)
                nc.gpsimd.tensor_tensor(
                    out=AnT[:], in0=AT[:], in1=rc[:].to_broadcast([128, ST, 128]),
                    op=ALU.mult,
                )

                # ax node-major: per-stack matmul with eT slice stationary
                ps_axn = ppool.tile([128, ST, 128], F32, tag="ps")
                for st in range(ST):
                    nc.tensor.matmul(ps_axn[:, st, :], eT[:, st, :], Wa[:])
                if has_ba:
                    for st in range(ST):
                        nc.vector.tensor_tensor(
                            out=ps_axn[:, st, :], in0=ps_axn[:, st, :],
                            in1=ba_full[:], op=ALU.add,
                        )
                axn = wpool.tile([128, ST, 128], BF, tag="axn" + s)
                nc.scalar.activation(axn[:], ps_axn[:], AF.Relu)

                # ux feature-major (weight-stationary big matmul)
                ps_ux = ppool.tile([128, ST, 128], F32, tag="ps")
                nc.tensor.matmul(ps_ux[:].rearrange("p s c -> p (s c)"), Wu[:],
                                 eT[:].rearrange("p s c -> p (s c)"))
                uxT = wpool.tile([128, ST, 128], BF, tag="uxT" + s)
                nc.scalar.activation(uxT[:], ps_ux[:], AF.Relu, bias=bu[:, 0:1])

                if STAGE == 1:
                    dd = dg1 if side == 1 else dg2
                    nc.sync.dma_start(out=dd[t * G:(t + 1) * G:2], in_=axn[0:1, :, :])
                    nc.sync.dma_start(out=dd[t * G + 1:(t + 1) * G:2], in_=axn[64:65, :, :])
                    return None, None

                # gconv: e^T = (An @ relu(ax))^T + relu(ux)^T
                ps_e = ppool.tile([128, ST, 128], F32, tag="ps")
                for st in range(ST):
                    nc.tensor.matmul(ps_e[:, st, :], axn[:, st, :], AnT[:, st, :])
                eTe = kpool.tile([128, ST, 128], BF, tag="eTe" + s)
                nc.vector.tensor_tensor(out=eTe[:], in0=ps_e[:], in1=uxT[:], op=ALU.add)

                # node-major copy for the z matmuls
                ps_en = ppool.tile([128, ST, 128], BF, tag="ps")
                for st in range(ST):
                    nc.tensor.transpose(ps_en[:, st, :], eTe[:, st, :], Ib[:])
                e_n = kpool.tile([128, ST, 128], BF, tag="en" + s)
                nc.vector.tensor_copy(e_n[:], ps_en[:])
                if STAGE == 2:
                    dd = dg1 if side == 1 else dg2
                    nc.sync.dma_start(out=dd[t * G:(t + 1) * G].rearrange("b d -> d b"),
                                      in_=eTe[:].rearrange("p s (pp n) -> p (s pp) n", pp=2)[:, :, 0])
                    return None, None
                return eTe, e_n

            def stage_a(t, dmas):
                e1T, e1n = phase_a_side(*dmas[1], t, 1)
                e2T, e2n = phase_a_side(*dmas[2], t, 2)
                if STAGE <= 2:
                    return None
                # t = emb1 @ Aff (feature-major)
                ps_tT = ppool.tile([128, ST, 128], F32, tag="ps")
                nc.tensor.matmul(ps_tT[:].rearrange("p s c -> p (s c)"), Aff[:],
                                 e1T[:].rearrange("p s c -> p (s c)"))
                tT = wpool.tile([128, ST, 128], BF, tag="tT")
                nc.scalar.copy(tT[:], ps_tT[:])
                return dict(e1T=e1T, e1n=e1n, e2T=e2T, e2n=e2n, tT=tT)

            def softmax_2side(pss, t):
                """Row softmax of both score blocks, side-interleaved for ILP."""
                mxns, sbs, Es, rss, sms = [], [], [], [], []
                for i, ps_x in enumerate(pss):
                    mxn = wpool.tile([128, ST], F32, tag=f"mxn{i}")
                    nc.vector.reduce_sum(mxn[:], ps_x[:], axis=AX.X, op=ALU.max, negate=True)
                    mxns.append(mxn)
                for i, ps_x in enumerate(pss):
                    sb = wpool.tile([128, ST, N], F32, tag=f"sb{i}")
                    nc.vector.tensor_tensor(out=sb[:], in0=ps_x[:],
                                            in1=mxns[i][:].to_broadcast([128, ST, N]),
                                            op=ALU.add)
                    sbs.append(sb)
                for i in range(2):
                    E = wpool.tile([128, ST, N], BF, tag=f"E{i}")
                    nc.scalar.activation(E[:], sbs[i][:], AF.Exp)
                    Es.append(E)
                for i in range(2):
                    den = wpool.tile([128, ST], F32, tag=f"den{i}")
                    nc.vector.reduce_sum(den[:], Es[i][:], axis=AX.X)
                    rs = wpool.tile([128, ST], F32, tag=f"rs{i}")
                    nc.vector.reciprocal(rs[:], den[:])
                    rss.append(rs)
                for i in range(2):
                    sm = zpool.tile([128, ST, 128], BF, tag=f"sm{i}")
                    if t < 2:
                        nc.gpsimd.memset(sm[:], 0.0)
                    sms.append(sm)
                for i in range(2):
                    for par in range(2):
                        sl = slice(par * 64, (par + 1) * 64)
                        nc.vector.tensor_tensor(
                            out=sms[i][sl, :, par * 64:(par + 1) * 64], in0=Es[i][sl, :, :],
                            in1=rss[i][sl, :].to_broadcast([64, ST, N]), op=ALU.mult,
                        )
                return sms

            def stage_b1(t, a):
                tTf = a["tT"][:].rearrange("p s c -> p (s c)")
                e2Tf = a["e2T"][:].rearrange("p s c -> p (s c)")

                ps_s = ppool.tile([128, ST, N], F32, tag="ps")
                ps_sT = ppool.tile([128, ST, N], F32, tag="ps")
                for b in range(G):
                    st, par = b // 2, b % 2
                    sl = slice(par * 64, (par + 1) * 64)
                    nc.tensor.matmul(ps_s[sl, st, :], tTf[:, b * N:(b + 1) * N],
                                     e2Tf[:, b * N:(b + 1) * N], tile_position=(0, par * 64))
                    nc.tensor.matmul(ps_sT[sl, st, :], e2Tf[:, b * N:(b + 1) * N],
                                     tTf[:, b * N:(b + 1) * N], tile_position=(0, par * 64))
                if STAGE == 3:
                    stmp = wpool.tile([128, ST, N], F32, tag="stmp")
                    nc.scalar.copy(stmp[:], ps_s[:])
                    nc.sync.dma_start(out=dg1[t * G:(t + 1) * G:2, 0:64], in_=stmp[0:1, :, :])
                    nc.sync.dma_start(out=dg1[t * G + 1:(t + 1) * G:2, 0:64], in_=stmp[64:65, :, :])
                    stmp2 = wpool.tile([128, ST, N], F32, tag="stmp2")
                    nc.scalar.copy(stmp2[:], ps_sT[:])
                    nc.sync.dma_start(out=dg2[t * G:(t + 1) * G:2, 0:64], in_=stmp2[0:1, :, :])
                    nc.sync.dma_start(out=dg2[t * G + 1:(t + 1) * G:2, 0:64], in_=stmp2[64:65, :, :])
                    return None

                sm1, sm2 = softmax_2side((ps_s, ps_sT), t)
                return dict(sm1=sm1, sm2=sm2)

            def stage_b2(t, b1):
                sm1, sm2 = b1["sm1"], b1["sm2"]
                ps_smT1 = ppool.tile([128, ST, 128], BF, tag="ps")
                ps_smT2 = ppool.tile([128, ST, 128], BF, tag="ps")
                for st in range(ST):
                    nc.tensor.transpose(ps_smT1[:, st, :], sm1[:, st, :], Ib[:])
                    nc.tensor.transpose(ps_smT2[:, st, :], sm2[:, st, :], Ib[:])
                sm1T = wpool.tile([128, ST, 128], BF, tag="sm1T")
                nc.scalar.copy(sm1T[:], ps_smT1[:])
                sm2T = wpool.tile([128, ST, 128], BF, tag="sm2T")
                nc.scalar.copy(sm2T[:], ps_smT2[:])
                if STAGE == 35:
                    nc.sync.dma_start(out=dg1[t * G:(t + 1) * G:2, 0:64], in_=sm1T[0:1, :, 0:64])
                    nc.sync.dma_start(out=dg1[t * G + 1:(t + 1) * G:2, 0:64], in_=sm1T[64:65, :, 64:128])
                    nc.sync.dma_start(out=dg2[t * G:(t + 1) * G:2, 0:64], in_=sm2T[0:1, :, 0:64])
                    nc.sync.dma_start(out=dg2[t * G + 1:(t + 1) * G:2, 0:64], in_=sm2T[64:65, :, 64:128])
                    return None
                return dict(sm1T=sm1T, sm2T=sm2T)

            def pool_side(nT, Wp, gacc, t, side):
                """SimGNN attention pooling, feature-major; sigmoid via tanh."""
                v8 = nT[:].rearrange("p s (pp n) -> p (s pp) n", pp=2)  # [128, 8, 64]
                msum = wpool.tile([128, G], F32, tag=f"msum{side}")
                nc.vector.reduce_sum(msum[:], v8, axis=AX.X)
                ps_ctx = spool.tile([128, G], F32, tag="s")
                nc.tensor.matmul(ps_ctx[:], Wp[:], msum[:])
                ctx = wpool.tile([128, G], BF, tag=f"ctx{side}")
                nc.scalar.activation(ctx[:], ps_ctx[:], AF.Tanh, scale=1.0 / N)
                tmp = wpool.tile([128, G, N], BF, tag=f"tmp{side}")
                nc.gpsimd.tensor_tensor(
                    out=tmp[:], in0=v8,
                    in1=ctx[:].to_broadcast([128, G, N]), op=ALU.mult,
                )
                ps_sc = spool.tile([1, G * N], F32, tag="s")
                nc.tensor.matmul(ps_sc[:], ones_col[:], tmp[:].rearrange("p g n -> p (g n)"))
                # tanh(y/2): sigmoid(y) = 0.5 + 0.5*tanh(y/2)
                tsc = wpool.tile([1, G * N], BF, tag=f"tsc{side}")
                nc.scalar.activation(tsc[:], ps_sc[:], AF.Tanh, scale=0.5)
                # replicate 0.5*tanh to all partitions via PE
                ps_r = spool.tile([128, G * N], F32, tag="s")
                nc.tensor.matmul(ps_r[:], half_row[:], tsc[:])
                tmp2 = wpool.tile([128, G, N], BF, tag=f"tmp2{side}")
                nc.vector.tensor_tensor(
                    out=tmp2[:], in0=v8,
                    in1=ps_r[:].rearrange("p (g n) -> p g n", g=G), op=ALU.mult,
                )
                gred = wpool.tile([128, G], F32, tag=f"gred{side}")
                nc.vector.reduce_sum(gred[:], tmp2[:], axis=AX.X)
                # g = 0.5*msum + gred  (one fused vector op into gacc slice)
                nc.vector.scalar_tensor_tensor(
                    out=gacc[:, t * G:(t + 1) * G], in0=msum[:], scalar=0.5,
                    in1=gred[:], op0=ALU.mult, op1=ALU.add,
                )

            def stage_c(t, a, bst):
                e1Tf = a["e1T"][:].rearrange("p s c -> p (s c)")
                e2Tf = a["e2T"][:].rearrange("p s c -> p (s c)")
                e1n, e2n = a["e1n"], a["e2n"]
                sm1T, sm2T = bst["sm1T"], bst["sm2T"]

                ps_z1 = ppool.tile([128, ST, 128], F32, tag="ps")
                ps_z2 = ppool.tile([128, ST, 128], F32, tag="ps")
                for st in range(ST):
                    nc.tensor.matmul(ps_z1[:, st, :], e2n[:, st, :], sm1T[:, st, :])
                    nc.tensor.matmul(ps_z2[:, st, :], e1n[:, st, :], sm2T[:, st, :])
                z1T = wpool.tile([128, ST, 128], BF, tag="z1T")
                nc.scalar.copy(z1T[:], ps_z1[:])
                z2T = wpool.tile([128, ST, 128], BF, tag="z2T")
                nc.scalar.copy(z2T[:], ps_z2[:])
                if STAGE == 4:
                    nc.sync.dma_start(out=dg1[t * G:(t + 1) * G].rearrange("b d -> d b"),
                                      in_=z1T[:].rearrange("p s (pp n) -> p (s pp) n", pp=2)[:, :, 0])
                    nc.sync.dma_start(out=dg2[t * G:(t + 1) * G].rearrange("b d -> d b"),
                                      in_=z2T[:].rearrange("p s (pp n) -> p (s pp) n", pp=2)[:, :, 0])
                    return

                ps_n1 = ppool.tile([128, ST, 128], F32, tag="ps")
                ps_n2 = ppool.tile([128, ST, 128], F32, tag="ps")
                nc.tensor.matmul(ps_n1[:].rearrange("p s c -> p (s c)"), Wct[:], e1Tf,
                                 start=True, stop=False)
                nc.tensor.matmul(ps_n1[:].rearrange("p s c -> p (s c)"), Wcb[:],
                                 z1T[:].rearrange("p s c -> p (s c)"), start=False, stop=True)
                nc.tensor.matmul(ps_n2[:].rearrange("p s c -> p (s c)"), Wct[:], e2Tf,
                                 start=True, stop=False)
                nc.tensor.matmul(ps_n2[:].rearrange("p s c -> p (s c)"), Wcb[:],
                                 z2T[:].rearrange("p s c -> p (s c)"), start=False, stop=True)
                n1T = wpool.tile([128, ST, 128], BF, tag="n1T")
                nc.scalar.activation(n1T[:], ps_n1[:], AF.Identity, bias=bc[:, 0:1])
                n2T = wpool.tile([128, ST, 128], BF, tag="n2T")
                nc.scalar.activation(n2T[:], ps_n2[:], AF.Identity, bias=bc[:, 0:1])
                if STAGE == 5:
                    nc.sync.dma_start(out=dg1[t * G:(t + 1) * G].rearrange("b d -> d b"),
                                      in_=n1T[:].rearrange("p s (pp n) -> p (s pp) n", pp=2)[:, :, 0])
                    nc.sync.dma_start(out=dg2[t * G:(t + 1) * G].rearrange("b d -> d b"),
                                      in_=n2T[:].rearrange("p s (pp n) -> p (s pp) n", pp=2)[:, :, 0])
                    return

                pool_side(n1T, Wp1, gacc1, t, 1)
                pool_side(n2T, Wp2, gacc2, t, 2)

            # ---- pipeline driver ----
            if STAGE <= 2:
                for t in range(NT):
                    stage_a(t, stage_a_dma(t))
            elif STAGE in (3, 35):
                a = {0: stage_a(0, stage_a_dma(0))}
                for t in range(NT):
                    if t + 1 < NT:
                        a[t + 1] = stage_a(t + 1, stage_a_dma(t + 1))
                    b1 = stage_b1(t, a.pop(t))
                    if STAGE == 35 and b1 is not None:
                        stage_b2(t, b1)
            else:
                # emission order per iteration: DMA(t+2) | C(t) | B1+B2(t+1) |
                # A-compute(t+2) -- each engine keeps ~1 tile of ready work.
                a = {0: stage_a(0, stage_a_dma(0))}
                if NT > 1:
                    a[1] = stage_a(1, stage_a_dma(1))
                bq = {0: stage_b2(0, stage_b1(0, a[0]))}
                for t in range(NT):
                    dm = stage_a_dma(t + 2) if t + 2 < NT else None
                    stage_c(t, a.pop(t), bq.pop(t))
                    if t + 1 < NT:
                        bq[t + 1] = stage_b2(t + 1, stage_b1(t + 1, a[t + 1]))
                    if dm is not None:
                        a[t + 2] = stage_a(t + 2, dm)

                # flush pooled outputs: transpose [128(d), n_pairs] -> rows
                for gacc, dg in ((gacc1, dg1), (gacc2, dg2)):
                    nb = 0
                    while nb < n_pairs:
                        w = min(128, n_pairs - nb)
                        ps_g = spool.tile([w, 128], F32, tag="s")
                        nc.tensor.transpose(ps_g[:], gacc[:, nb:nb + w], I[:])
                        gout = wpool.tile([w, 128], F32, tag="gout")
                        nc.scalar.copy(gout[:], ps_g[:])
                        nc.sync.dma_start(out=dg[nb:nb + w], in_=gout[:])
                        nb += w
    nc.finalize()
    return nc


_BUILT = {}


def _get_nc(n_pairs, has_ba=False):
    key = (n_pairs, has_ba)
    if key not in _BUILT:
        nc = bacc.Bacc("TRN2", target_bir_lowering=False, debug=False,
                       num_devices=NCORES)
        _BUILT[key] = _emit(nc, n_pairs, has_ba)
    return _BUILT[key]


def kernel(A_src, emb_src, mask_src, A_dst, emb_dst, mask_dst,
           Wa, ba, Wu, bu, Aff, Wc, bc, Wp1, Wp2):
    import ml_dtypes
    bf = ml_dtypes.bfloat16

    A_src = np.asarray(A_src, dtype=np.float32)
    A_dst = np.asarray(A_dst, dtype=np.float32)
    emb_src = np.asarray(emb_src, dtype=np.float32)
    emb_dst = np.asarray(emb_dst, dtype=np.float32)
    ba = np.asarray(ba, np.float32)
    n_total = A_src.shape[0]
    n_pairs = n_total // NCORES
    has_ba = bool(np.any(ba))
    nc = _get_nc(n_pairs, has_ba)

    # host-side layout prep (transpose + bf16); pure data movement
    ATs = np.ascontiguousarray(A_src.transpose(0, 2, 1)).astype(bf)
    ATd = np.ascontiguousarray(A_dst.transpose(0, 2, 1)).astype(bf)
    # tiled feature-major embeddings: [NT_total, 128, G*64]
    ETs = np.ascontiguousarray(
        emb_src.reshape(n_total // G, G, N, D).transpose(0, 3, 1, 2)
        .reshape(n_total // G, D, G * N)).astype(bf)
    ETd = np.ascontiguousarray(
        emb_dst.reshape(n_total // G, G, N, D).transpose(0, 3, 1, 2)
        .reshape(n_total // G, D, G * N)).astype(bf)

    Wc = np.asarray(Wc, np.float32)
    shared = {
        "Wa": np.asarray(Wa, np.float32).astype(bf),
        "Wu": np.asarray(Wu, np.float32).astype(bf),
        "Aff": np.asarray(Aff, np.float32).astype(bf),
        "Wct": np.ascontiguousarray(Wc[:D]).astype(bf),
        "Wcb": np.ascontiguousarray(Wc[D:]).astype(bf),
        "Wp1": np.asarray(Wp1, np.float32),
        "Wp2": np.asarray(Wp2, np.float32),
        "bu_col": np.ascontiguousarray(np.asarray(bu, np.float32)[:, None]),
        "bc_col": np.ascontiguousarray(np.asarray(bc, np.float32)[:, None]),
        "ba_row": np.ascontiguousarray(ba[None, :]),
        "ident_bf": np.eye(128, dtype=bf),
        "ident": np.eye(128, dtype=np.float32),
    }
    NTc = n_pairs // G
    in_maps = []
    for c in range(NCORES):
        sl = slice(c * n_pairs, (c + 1) * n_pairs)
        slt = slice(c * NTc, (c + 1) * NTc)
        in_maps.append({
            "AT_src": ATs[sl], "AT_dst": ATd[sl],
            "embT_src": ETs[slt], "embT_dst": ETd[slt],
            **shared,
        })
    res = run_bass_kernel_spmd(nc, in_maps, list(range(NCORES)))
    g1 = np.concatenate([res.results[c]["g1"] for c in range(NCORES)], axis=0)
    g2 = np.concatenate([res.results[c]["g2"] for c in range(NCORES)], axis=0)
    return (g1, g2)


# revision 24
# speedup vs baseline: 1.0375x; 1.0375x over previous
"""Trainium2 Bass kernel for nn_CGFA (cross-graph feature aggregation / graph matching).

Pure data parallel over 8 NeuronCores: batch B=4096 -> 512 pairs per core.
Per core the batch is processed in tiles of G=8 pairs (4 stacks of 2 pairs,
64 nodes each, parity-packed on the 128 partitions).

Host-side prep (layout/dtype only): emb is uploaded pre-transposed
(feature-major) and A pre-transposed, both bf16, so the kernel never
transposes inputs on the PE.

3-stage software pipeline: A(t+2) loads+gconv | B(t+1) affinity+softmax |
C(t) cross-graph update + pooling, so every engine keeps ~1 tile of slack.

Only activation funcs from the shared "exp_and_others" table are used
(Exp/Relu/Tanh/Identity/Copy) to avoid ACT_TABLE_LOAD thrash; sigmoid is
computed as 0.5 + 0.5*tanh(x/2) with the affine folded into the pooling
combine.
"""

import os
import sys

STAGE = int(os.environ.get("CGFA_STAGE", "6"))

sys.path.insert(0, "/opt/trn_rl_repo")

import numpy as np

from concourse import bass, bacc
import concourse.mybir as mybir
from concourse.bass_utils import run_bass_kernel_spmd
from concourse.tile import TileContext

F32 = mybir.dt.float32
BF = mybir.dt.bfloat16
AF = mybir.ActivationFunctionType
ALU = mybir.AluOpType
AX = mybir.AxisListType

B, N, D = 4096, 64, 128
NCORES = 8
G = 8          # pairs per tile
ST = G // 2    # 128-partition stacks per side


def _emit(nc, n_pairs, has_ba=False):
    NT = n_pairs // G

    # ---- DRAM I/O ----
    dET1 = nc.dram_tensor("embT_src", [NT, D, G * N], BF, kind="ExternalInput").ap()
    dET2 = nc.dram_tensor("embT_dst", [NT, D, G * N], BF, kind="ExternalInput").ap()
    dAT1 = nc.dram_tensor("AT_src", [n_pairs, N, N], BF, kind="ExternalInput").ap()
    dAT2 = nc.dram_tensor("AT_dst", [n_pairs, N, N], BF, kind="ExternalInput").ap()
    dWa = nc.dram_tensor("Wa", [D, D], BF, kind="ExternalInput").ap()
    dWu = nc.dram_tensor("Wu", [D, D], BF, kind="ExternalInput").ap()
    dAff = nc.dram_tensor("Aff", [D, D], BF, kind="ExternalInput").ap()
    dWct = nc.dram_tensor("Wct", [D, D], BF, kind="ExternalInput").ap()
    dWcb = nc.dram_tensor("Wcb", [D, D], BF, kind="ExternalInput").ap()
    dWp1 = nc.dram_tensor("Wp1", [D, D], F32, kind="ExternalInput").ap()
    dWp2 = nc.dram_tensor("Wp2", [D, D], F32, kind="ExternalInput").ap()
    dbu = nc.dram_tensor("bu_col", [D, 1], F32, kind="ExternalInput").ap()
    dbc = nc.dram_tensor("bc_col", [D, 1], F32, kind="ExternalInput").ap()
    dba = nc.dram_tensor("ba_row", [1, D], F32, kind="ExternalInput").ap()
    dIb = nc.dram_tensor("ident_bf", [128, 128], BF, kind="ExternalInput").ap()
    dI = nc.dram_tensor("ident", [128, 128], F32, kind="ExternalInput").ap()
    dg1 = nc.dram_tensor("g1", [n_pairs, D], F32, kind="ExternalOutput").ap()
    dg2 = nc.dram_tensor("g2", [n_pairs, D], F32, kind="ExternalOutput").ap()

    with TileContext(nc) as tc:
        with (
            tc.tile_pool(name="const", bufs=1) as cpool,
            tc.tile_pool(name="work", bufs=3) as wpool,
            tc.tile_pool(name="keep", bufs=4) as kpool,     # A-outputs live A(t)..C(t)
            tc.tile_pool(name="zb", bufs=2) as zpool,       # pre-zeroed block-diag tiles
            tc.tile_pool(name="psum", bufs=6, space="PSUM") as ppool,
            tc.tile_pool(name="psums", bufs=2, space="PSUM") as spool,
        ):
            # ---- constants ----
            Wa = cpool.tile([128, 128], BF, tag="Wa")
            Wu = cpool.tile([128, 128], BF, tag="Wu")
            Aff = cpool.tile([128, 128], BF, tag="Aff")
            Wct = cpool.tile([128, 128], BF, tag="Wct")
            Wcb = cpool.tile([128, 128], BF, tag="Wcb")
            Wp1 = cpool.tile([128, 128], F32, tag="Wp1")
            Wp2 = cpool.tile([128, 128], F32, tag="Wp2")
            Ib = cpool.tile([128, 128], BF, tag="Ib")
            I = cpool.tile([128, 128], F32, tag="I")
            bu = cpool.tile([128, 1], F32, tag="bu")
            bc = cpool.tile([128, 1], F32, tag="bc")
            ones_col = cpool.tile([128, 1], BF, tag="ones_col")
            half_row = cpool.tile([1, 128], BF, tag="half_row")
            for tile_, src in (
                (Wa, dWa), (Wu, dWu), (Aff, dAff), (Wct, dWct), (Wcb, dWcb),
                (Wp1, dWp1), (Wp2, dWp2), (Ib, dIb), (I, dI), (bu, dbu), (bc, dbc),
            ):
                nc.sync.dma_start(out=tile_[:], in_=src)
            nc.gpsimd.memset(ones_col[:], 1.0)
            nc.gpsimd.memset(half_row[:], 0.5)
            if has_ba:
                ba_row = cpool.tile([1, 128], F32, tag="ba_row")
                nc.sync.dma_start(out=ba_row[:], in_=dba)
                ba_full = cpool.tile([128, 128], F32, tag="ba_full")
                nc.gpsimd.partition_broadcast(ba_full[:], ba_row[:])

            # whole-core pooled-output accumulators (written tile by tile)
            gacc1 = cpool.tile([128, n_pairs], F32, tag="gacc1")
            gacc2 = cpool.tile([128, n_pairs], F32, tag="gacc2")

            def stage_a_dma(t):
                """Issue all of tile t's input DMAs early."""
                out = {}
                for side, dET, dAT in ((1, dET1, dAT1), (2, dET2, dAT2)):
                    s = str(side)
                    eT = wpool.tile([128, ST, 128], BF, tag="eT" + s)
                    nc.sync.dma_start(out=eT[:], in_=dET[t].rearrange("d (s c) -> d s c", s=ST))
                    AT = zpool.tile([128, ST, 128], BF, tag="AT" + s)
                    if t < 2:
                        nc.gpsimd.memset(AT[:], 0.0)
                    nc.sync.dma_start(
                        out=AT[0:64, :, 0:64],
                        in_=dAT[t * G:(t + 1) * G:2].rearrange("g j i -> j g i"),
                    )
                    nc.sync.dma_start(
                        out=AT[64:128, :, 64:128],
                        in_=dAT[t * G + 1:(t + 1) * G:2].rearrange("g j i -> j g i"),
                    )
                    out[side] = (eT, AT)
                return out

            def phase_a_side(eT, AT, t, side):
                """gconv for one side; returns (eTe, e_n)."""
                s = str(side)
                # column sums of A (= row sums of A^T) -> 1/c, fold into A^T
                c = wpool.tile([128, ST], F32, tag="c" + s)
                nc.vector.reduce_sum(c[:], AT[:], axis=AX.X)
                nc.vector.tensor_scalar_max(c[:], c[:], 1e-12)
                rc = wpool.tile([128, ST], F32, tag="rc" + s)
                nc.vector.reciprocal(rc[:], c[:])
                AnT = wpool.tile([128, ST, 128], BF, tag="AnT" + s)
                nc.gpsimd.tensor_tensor(
                    out=AnT[:], in0=AT[:], in1=rc[:].to_broadcast([128, ST, 128]),
                    op=ALU.mult,
                )

                # ax node-major: per-stack matmul with eT slice stationary
                ps_axn = ppool.tile([128, ST, 128], F32, tag="ps")
                for st in range(ST):
                    nc.tensor.matmul(ps_axn[:, st, :], eT[:, st, :], Wa[:])
                if has_ba:
                    for st in range(ST):
                        nc.vector.tensor_tensor(
                            out=ps_axn[:, st, :], in0=ps_axn[:, st, :],
                            in1=ba_full[:], op=ALU.add,
                        )
                axn = wpool.tile([128, ST, 128], BF, tag="axn" + s)
                nc.scalar.activation(axn[:], ps_axn[:], AF.Relu)

                # ux feature-major (weight-stationary big matmul)
                ps_ux = ppool.tile([128, ST, 128], F32, tag="ps")
                nc.tensor.matmul(ps_ux[:].rearrange("p s c -> p (s c)"), Wu[:],
                                 eT[:].rearrange("p s c -> p (s c)"))
                uxT = wpool.tile([128, ST, 128], BF, tag="uxT" + s)
                nc.scalar.activation(uxT[:], ps_ux[:], AF.Relu, bias=bu[:, 0:1])

                if STAGE == 1:
                    dd = dg1 if side == 1 else dg2
                    nc.sync.dma_start(out=dd[t * G:(t + 1) * G:2], in_=axn[0:1, :, :])
                    nc.sync.dma_start(out=dd[t * G + 1:(t + 1) * G:2], in_=axn[64:65, :, :])
                    return None, None

                # gconv: e^T = (An @ relu(ax))^T + relu(ux)^T
                ps_e = ppool.tile([128, ST, 128], F32, tag="ps")
                for st in range(ST):
                    nc.tensor.matmul(ps_e[:, st, :], axn[:, st, :], AnT[:, st, :])
                eTe = kpool.tile([128, ST, 128], BF, tag="eTe" + s)
                nc.vector.tensor_tensor(out=eTe[:], in0=ps_e[:], in1=uxT[:], op=ALU.add)

                # node-major copy for the z matmuls
                ps_en = ppool.tile([128, ST, 128], BF, tag="ps")
                for st in range(ST):
                    nc.tensor.transpose(ps_en[:, st, :], eTe[:, st, :], Ib[:])
                e_n = kpool.tile([128, ST, 128], BF, tag="en" + s)
                nc.vector.tensor_copy(e_n[:], ps_en[:])
                if STAGE == 2:
                    dd = dg1 if side == 1 else dg2
                    nc.sync.dma_start(out=dd[t * G:(t + 1) * G].rearrange("b d -> d b"),
                                      in_=eTe[:].rearrange("p s (pp n) -> p (s pp) n", pp=2)[:, :, 0])
                    return None, None
                return eTe, e_n

            def stage_a(t, dmas):
                e1T, e1n = phase_a_side(*dmas[1], t, 1)
                e2T, e2n = phase_a_side(*dmas[2], t, 2)
                if STAGE <= 2:
                    return None
                # t = emb1 @ Aff (feature-major)
                ps_tT = ppool.tile([128, ST, 128], F32, tag="ps")
                nc.tensor.matmul(ps_tT[:].rearrange("p s c -> p (s c)"), Aff[:],
                                 e1T[:].rearrange("p s c -> p (s c)"))
                tT = wpool.tile([128, ST, 128], BF, tag="tT")
                nc.scalar.copy(tT[:], ps_tT[:])
                return dict(e1T=e1T, e1n=e1n, e2T=e2T, e2n=e2n, tT=tT)

            def softmax_2side(pss, t):
                """Row softmax of both score blocks, side-interleaved for ILP.
                Max-subtract is fused into per-stack Exp via per-partition bias."""
                mxns, Es, rss, sms = [], [], [], []
                for i, ps_x in enumerate(pss):
                    mxn = wpool.tile([128, ST], F32, tag=f"mxn{i}")
                    nc.vector.reduce_sum(mxn[:], ps_x[:], axis=AX.X, op=ALU.max, negate=True)
                    mxns.append(mxn)
                for i, ps_x in enumerate(pss):
                    E = wpool.tile([128, ST, N], BF, tag=f"E{i}")
                    for st in range(ST):
                        nc.scalar.activation(E[:, st, :], ps_x[:, st, :], AF.Exp,
                                             bias=mxns[i][:, st:st + 1])
                    Es.append(E)
                for i in range(2):
                    den = wpool.tile([128, ST], F32, tag=f"den{i}")
                    nc.vector.reduce_sum(den[:], Es[i][:], axis=AX.X)
                    rs = wpool.tile([128, ST], F32, tag=f"rs{i}")
                    nc.vector.reciprocal(rs[:], den[:])
                    rss.append(rs)
                for i in range(2):
                    sm = zpool.tile([128, ST, 128], BF, tag=f"sm{i}")
                    if t < 2:
                        nc.gpsimd.memset(sm[:], 0.0)
                    sms.append(sm)
                for i in range(2):
                    for par in range(2):
                        sl = slice(par * 64, (par + 1) * 64)
                        nc.gpsimd.tensor_tensor(
                            out=sms[i][sl, :, par * 64:(par + 1) * 64], in0=Es[i][sl, :, :],
                            in1=rss[i][sl, :].to_broadcast([64, ST, N]), op=ALU.mult,
                        )
                return sms

            def stage_b1(t, a):
                tTf = a["tT"][:].rearrange("p s c -> p (s c)")
                e2Tf = a["e2T"][:].rearrange("p s c -> p (s c)")

                ps_s = ppool.tile([128, ST, N], F32, tag="ps")
                ps_sT = ppool.tile([128, ST, N], F32, tag="ps")
                for b in range(G):
                    st, par = b // 2, b % 2
                    sl = slice(par * 64, (par + 1) * 64)
                    nc.tensor.matmul(ps_s[sl, st, :], tTf[:, b * N:(b + 1) * N],
                                     e2Tf[:, b * N:(b + 1) * N], tile_position=(0, par * 64))
                    nc.tensor.matmul(ps_sT[sl, st, :], e2Tf[:, b * N:(b + 1) * N],
                                     tTf[:, b * N:(b + 1) * N], tile_position=(0, par * 64))
                if STAGE == 3:
                    stmp = wpool.tile([128, ST, N], F32, tag="stmp")
                    nc.scalar.copy(stmp[:], ps_s[:])
                    nc.sync.dma_start(out=dg1[t * G:(t + 1) * G:2, 0:64], in_=stmp[0:1, :, :])
                    nc.sync.dma_start(out=dg1[t * G + 1:(t + 1) * G:2, 0:64], in_=stmp[64:65, :, :])
                    stmp2 = wpool.tile([128, ST, N], F32, tag="stmp2")
                    nc.scalar.copy(stmp2[:], ps_sT[:])
                    nc.sync.dma_start(out=dg2[t * G:(t + 1) * G:2, 0:64], in_=stmp2[0:1, :, :])
                    nc.sync.dma_start(out=dg2[t * G + 1:(t + 1) * G:2, 0:64], in_=stmp2[64:65, :, :])
                    return None

                sm1, sm2 = softmax_2side((ps_s, ps_sT), t)
                return dict(sm1=sm1, sm2=sm2)

            def stage_b2(t, b1):
                sm1, sm2 = b1["sm1"], b1["sm2"]
                ps_smT1 = ppool.tile([128, ST, 128], BF, tag="ps")
                ps_smT2 = ppool.tile([128, ST, 128], BF, tag="ps")
                for st in range(ST):
                    nc.tensor.transpose(ps_smT1[:, st, :], sm1[:, st, :], Ib[:])
                    nc.tensor.transpose(ps_smT2[:, st, :], sm2[:, st, :], Ib[:])
                sm1T = wpool.tile([128, ST, 128], BF, tag="sm1T")
                nc.scalar.copy(sm1T[:], ps_smT1[:])
                sm2T = wpool.tile([128, ST, 128], BF, tag="sm2T")
                nc.scalar.copy(sm2T[:], ps_smT2[:])
                if STAGE == 35:
                    nc.sync.dma_start(out=dg1[t * G:(t + 1) * G:2, 0:64], in_=sm1T[0:1, :, 0:64])
                    nc.sync.dma_start(out=dg1[t * G + 1:(t + 1) * G:2, 0:64], in_=sm1T[64:65, :, 64:128])
                    nc.sync.dma_start(out=dg2[t * G:(t + 1) * G:2, 0:64], in_=sm2T[0:1, :, 0:64])
                    nc.sync.dma_start(out=dg2[t * G + 1:(t + 1) * G:2, 0:64], in_=sm2T[64:65, :, 64:128])
                    return None
                return dict(sm1T=sm1T, sm2T=sm2T)

            def stage_p(t, c):
                """SimGNN attention pooling, both sides interleaved; sigmoid
                via tanh: sigma(y) = 0.5 + 0.5*tanh(y/2)."""
                v8s, msums, ctxs, tmps, tscs, rpls = [], [], [], [], [], []
                for side, nT in ((1, c["n1T"]), (2, c["n2T"])):
                    v8s.append(nT[:].rearrange("p s (pp n) -> p (s pp) n", pp=2))
                for side in range(2):
                    msum = wpool.tile([128, G], F32, tag=f"msum{side}")
                    nc.vector.reduce_sum(msum[:], v8s[side], axis=AX.X)
                    msums.append(msum)
                for side, Wp in enumerate((Wp1, Wp2)):
                    ps_ctx = spool.tile([128, G], F32, tag="s")
                    nc.tensor.matmul(ps_ctx[:], Wp[:], msums[side][:])
                    ctx = wpool.tile([128, G], BF, tag=f"ctx{side}")
                    nc.scalar.activation(ctx[:], ps_ctx[:], AF.Tanh, scale=1.0 / N)
                    ctxs.append(ctx)
                for side in range(2):
                    tmp = wpool.tile([128, G, N], BF, tag=f"tmp{side}")
                    nc.gpsimd.tensor_tensor(
                        out=tmp[:], in0=v8s[side],
                        in1=ctxs[side][:].to_broadcast([128, G, N]), op=ALU.mult,
                    )
                    tmps.append(tmp)
                for side in range(2):
                    ps_sc = spool.tile([1, G * N], F32, tag="s")
                    nc.tensor.matmul(ps_sc[:], ones_col[:],
                                     tmps[side][:].rearrange("p g n -> p (g n)"))
                    tsc = wpool.tile([1, G * N], BF, tag=f"tsc{side}")
                    nc.scalar.activation(tsc[:], ps_sc[:], AF.Tanh, scale=0.5)
                    tscs.append(tsc)
                for side in range(2):
                    ps_r = ppool.tile([128, G * N], F32, tag="ps")
                    nc.tensor.matmul(ps_r[:], half_row[:], tscs[side][:])
                    rpls.append(ps_r)
                for side, gacc in enumerate((gacc1, gacc2)):
                    tmp2 = wpool.tile([128, G, N], BF, tag=f"tmp2{side}")
                    nc.vector.tensor_tensor(
                        out=tmp2[:], in0=v8s[side],
                        in1=rpls[side][:].rearrange("p (g n) -> p g n", g=G), op=ALU.mult,
                    )
                    gred = wpool.tile([128, G], F32, tag=f"gred{side}")
                    nc.vector.reduce_sum(gred[:], tmp2[:], axis=AX.X)
                    nc.vector.scalar_tensor_tensor(
                        out=gacc[:, t * G:(t + 1) * G], in0=msums[side][:], scalar=0.5,
                        in1=gred[:], op0=ALU.mult, op1=ALU.add,
                    )

            def stage_c(t, a, bst):
                e1Tf = a["e1T"][:].rearrange("p s c -> p (s c)")
                e2Tf = a["e2T"][:].rearrange("p s c -> p (s c)")
                e1n, e2n = a["e1n"], a["e2n"]
                sm1T, sm2T = bst["sm1T"], bst["sm2T"]

                ps_z1 = ppool.tile([128, ST, 128], F32, tag="ps")
                ps_z2 = ppool.tile([128, ST, 128], F32, tag="ps")
                for st in range(ST):
                    nc.tensor.matmul(ps_z1[:, st, :], e2n[:, st, :], sm1T[:, st, :])
                    nc.tensor.matmul(ps_z2[:, st, :], e1n[:, st, :], sm2T[:, st, :])
                z1T = wpool.tile([128, ST, 128], BF, tag="z1T")
                nc.scalar.copy(z1T[:], ps_z1[:])
                z2T = wpool.tile([128, ST, 128], BF, tag="z2T")
                nc.scalar.copy(z2T[:], ps_z2[:])
                if STAGE == 4:
                    nc.sync.dma_start(out=dg1[t * G:(t + 1) * G].rearrange("b d -> d b"),
                                      in_=z1T[:].rearrange("p s (pp n) -> p (s pp) n", pp=2)[:, :, 0])
                    nc.sync.dma_start(out=dg2[t * G:(t + 1) * G].rearrange("b d -> d b"),
                                      in_=z2T[:].rearrange("p s (pp n) -> p (s pp) n", pp=2)[:, :, 0])
                    return

                ps_n1 = ppool.tile([128, ST, 128], F32, tag="ps")
                ps_n2 = ppool.tile([128, ST, 128], F32, tag="ps")
                nc.tensor.matmul(ps_n1[:].rearrange("p s c -> p (s c)"), Wct[:], e1Tf,
                                 start=True, stop=False)
                nc.tensor.matmul(ps_n1[:].rearrange("p s c -> p (s c)"), Wcb[:],
                                 z1T[:].rearrange("p s c -> p (s c)"), start=False, stop=True)
                nc.tensor.matmul(ps_n2[:].rearrange("p s c -> p (s c)"), Wct[:], e2Tf,
                                 start=True, stop=False)
                nc.tensor.matmul(ps_n2[:].rearrange("p s c -> p (s c)"), Wcb[:],
                                 z2T[:].rearrange("p s c -> p (s c)"), start=False, stop=True)
                n1T = wpool.tile([128, ST, 128], BF, tag="n1T")
                nc.scalar.activation(n1T[:], ps_n1[:], AF.Identity, bias=bc[:, 0:1])
                n2T = wpool.tile([128, ST, 128], BF, tag="n2T")
                nc.scalar.activation(n2T[:], ps_n2[:], AF.Identity, bias=bc[:, 0:1])
                if STAGE == 5:
                    nc.sync.dma_start(out=dg1[t * G:(t + 1) * G].rearrange("b d -> d b"),
                                      in_=n1T[:].rearrange("p s (pp n) -> p (s pp) n", pp=2)[:, :, 0])
                    nc.sync.dma_start(out=dg2[t * G:(t + 1) * G].rearrange("b d -> d b"),
                                      in_=n2T[:].rearrange("p s (pp n) -> p (s pp) n", pp=2)[:, :, 0])
                    return None
                return dict(n1T=n1T, n2T=n2T)

            # ---- pipeline driver ----
            if STAGE <= 2:
                for t in range(NT):
                    stage_a(t, stage_a_dma(t))
            elif STAGE in (3, 35):
                a = {0: stage_a(0, stage_a_dma(0))}
                for t in range(NT):
                    if t + 1 < NT:
                        a[t + 1] = stage_a(t + 1, stage_a_dma(t + 1))
                    b1 = stage_b1(t, a.pop(t))
                    if STAGE == 35 and b1 is not None:
                        stage_b2(t, b1)
            else:
                # 4-stage pipeline; emission order per iteration:
                # DMA(t+2) | P(t-1) | C(t) | B1+B2(t+1) | A-compute(t+2)
                # -- each engine keeps ~1 tile of ready work.
                a = {0: stage_a(0, stage_a_dma(0))}
                if NT > 1:
                    a[1] = stage_a(1, stage_a_dma(1))
                bq = {0: stage_b2(0, stage_b1(0, a[0]))}
                cq = {}
                for t in range(NT):
                    dm = stage_a_dma(t + 2) if t + 2 < NT else None
                    if cq.get(t - 1) is not None:
                        stage_p(t - 1, cq.pop(t - 1))
                    cq[t] = stage_c(t, a.pop(t), bq.pop(t))
                    if t + 1 < NT:
                        bq[t + 1] = stage_b2(t + 1, stage_b1(t + 1, a[t + 1]))
                    if dm is not None:
                        a[t + 2] = stage_a(t + 2, dm)
                if NT - 1 in cq and cq[NT - 1] is not None:
                    stage_p(NT - 1, cq.pop(NT - 1))

                # flush pooled outputs: transpose [128(d), n_pairs] -> rows
                for gacc, dg in ((gacc1, dg1), (gacc2, dg2)):
                    nb = 0
                    while nb < n_pairs:
                        w = min(128, n_pairs - nb)
                        ps_g = spool.tile([w, 128], F32, tag="s")
                        nc.tensor.transpose(ps_g[:], gacc[:, nb:nb + w], I[:])
                        gout = wpool.tile([w, 128], F32, tag="gout")
                        nc.scalar.copy(gout[:], ps_g[:])
                        nc.sync.dma_start(out=dg[nb:nb + w], in_=gout[:])
                        nb += w
    nc.finalize()
    return nc


_BUILT = {}


def _get_nc(n_pairs, has_ba=False):
    key = (n_pairs, has_ba)
    if key not in _BUILT:
        nc = bacc.Bacc("TRN2", target_bir_lowering=False, debug=False,
                       num_devices=NCORES)
        _BUILT[key] = _emit(nc, n_pairs, has_ba)
    return _BUILT[key]


def kernel(A_src, emb_src, mask_src, A_dst, emb_dst, mask_dst,
           Wa, ba, Wu, bu, Aff, Wc, bc, Wp1, Wp2):
    import ml_dtypes
    bf = ml_dtypes.bfloat16

    A_src = np.asarray(A_src, dtype=np.float32)
    A_dst = np.asarray(A_dst, dtype=np.float32)
    emb_src = np.asarray(emb_src, dtype=np.float32)
    emb_dst = np.asarray(emb_dst, dtype=np.float32)
    ba = np.asarray(ba, np.float32)
    n_total = A_src.shape[0]
    n_pairs = n_total // NCORES
    has_ba = bool(np.any(ba))
    nc = _get_nc(n_pairs, has_ba)

    # host-side layout prep (transpose + bf16); pure data movement
    ATs = np.ascontiguousarray(A_src.transpose(0, 2, 1)).astype(bf)
    ATd = np.ascontiguousarray(A_dst.transpose(0, 2, 1)).astype(bf)
    # tiled feature-major embeddings: [NT_total, 128, G*64]
    ETs = np.ascontiguousarray(
        emb_src.reshape(n_total // G, G, N, D).transpose(0, 3, 1, 2)
        .reshape(n_total // G, D, G * N)).astype(bf)
    ETd = np.ascontiguousarray(
        emb_dst.reshape(n_total // G, G, N, D).transpose(0, 3, 1, 2)
        .reshape(n_total // G, D, G * N)).astype(bf)

    Wc = np.asarray(Wc, np.float32)
    shared = {
        "Wa": np.asarray(Wa, np.float32).astype(bf),
        "Wu": np.asarray(Wu, np.float32).astype(bf),
        "Aff": np.asarray(Aff, np.float32).astype(bf),
        "Wct": np.ascontiguousarray(Wc[:D]).astype(bf),
        "Wcb": np.ascontiguousarray(Wc[D:]).astype(bf),
        "Wp1": np.asarray(Wp1, np.float32),
        "Wp2": np.asarray(Wp2, np.float32),
        "bu_col": np.ascontiguousarray(np.asarray(bu, np.float32)[:, None]),
        "bc_col": np.ascontiguousarray(np.asarray(bc, np.float32)[:, None]),
        "ba_row": np.ascontiguousarray(ba[None, :]),
        "ident_bf": np.eye(128, dtype=bf),
        "ident": np.eye(128, dtype=np.float32),
    }
    NTc = n_pairs // G
    in_maps = []
    for c in range(NCORES):
        sl = slice(c * n_pairs, (c + 1) * n_pairs)
        slt = slice(c * NTc, (c + 1) * NTc)
        in_maps.append({
            "AT_src": ATs[sl], "AT_dst": ATd[sl],
            "embT_src": ETs[slt], "embT_dst": ETd[slt],
            **shared,
        })
    res = run_bass_kernel_spmd(nc, in_maps, list(range(NCORES)))
    g1 = np.concatenate([res.results[c]["g1"] for c in range(NCORES)], axis=0)
    g2 = np.concatenate([res.results[c]["g2"] for c in range(NCORES)], axis=0)
    return (g1, g2)
